# revision 40
# baseline (speedup 1.0000x reference)
"""Dot-product stereo cost volume on 8 Trainium2 NeuronCores.

cost[b, d, y, x] = sum_c left[b,c,y,x] * right[b,c,y,x-d], zeros where x-d < 0.
Shapes: left/right [4, 128, 192, 640] fp32, D = 96 -> out [4, 96, 192, 640] fp32.

Strategy
--------
Sharding: 8 cores <- (b, y-half): core k handles batch k//2, rows 96*(k%2)..+96.
No halo needed (disparity shifts are along W only).

Per (y) row the math is a banded Gram matrix: G_y[x', x] = sum_c R[c,x'] L[c,x],
and cost[d, y, x] = G_y[x-d, x].  The PE computes G in M-row tiles:
tile t covers x' in [M*t, M*t+M), x in [M*t, M*t+M+96) (since d <= 95, every
needed (x', x) pair with x' in that M-block satisfies 0 <= x - x' <= M+95).
128//M such tiles stack into one [128, M+96] PSUM tile via tile_position column
groups.  The raw rect tiles stream to a DRAM scratch buffer in float16 (the
quantization error is relative to each stored value, ~2^-12, far inside the
2e-2 gate); the diagonal reindex (d = x - x') is absorbed into the host-side
unshard with one precomputed fancy index (a diagonal of G is not expressible
as a DMA access pattern: SBUF-side APs cannot couple partition and byte
offsets, and burst contiguity runs along d on the source but along x in the
output layout).

Engine budget per core (cost-model units, 2.4 GHz, 360 B/ns aggregate DMA):
loads 62.9 MB fp32 = 175 us (irreducible at fp32-exact precision); fp16
scratch in the M=64-rect layout is 19.7 MB -> DMA ~229 us total.  fp32
matmul streams at 4 cyc/row, so at M=64 the PE (256 us) binds instead.

The shipped mode "t64" decouples the two: the PE runs M=128 tiles (5 tiles
of [128, 224] per row -> 179 us, half the rhs restreaming of M=64, still
plain fp32 matmuls = baseline numerics), and each [128, 224] PSUM tile is
evacuated as two 64-row bands with a 64-col relative shift -- DVE takes
rows 0-63 cols [0,160), ACT rows 64-127 cols [64,224) -- which lands
exactly the M=64-rect scratch layout (a full per-partition shear is not
expressible in any single engine op or DMA access pattern; 32-row bands
would cut junk further but need a third PSUM-capable engine, and GPSIMD
cannot read PSUM).  The last col block stores 128 wide instead of 160
(x >= W junk cropped, uniform host index).  Queue discipline matters: SP
issues loads only, and ACT issues the store after its own band copies; any
arrangement where a sequencer's copy waits sit ahead of load dma_starts
idles the DMA pool ~10-40% (in-order sequencers park on semaphore waits).

Timeline-sim: 234.7 us vs 289.8 us for the original fp32-scratch baseline
(DMA busy 227 us at 97%, PE 184 us, DVE 153 us, ACT 154 us).  Other modes
(base/m128/split3/t32) are kept for reference; split3 (fp16 hi/lo 3-pass
PE at M=64, fp32-grade via G = r_hi l_hi + r_hi l_lo + r_lo l_hi) is the
fallback if fp32 M=128 matmuls ever misbehave, measured rel err 7.5e-3.

Session 2 findings (HW-measured via high-R repeat slope):
- Loads-only probe: 209.1 us +- 2.4 (sim 181.6) -> effective DMA bandwidth
  is ~307 B/ns, 0.85x the cost model's 360 (the model's own DMA_UTILIZATION
  = 0.83 fudge, absent from its descriptor-level bus estimate).  The kernel
  is DMA-BYTE-bound: t64's 81.8MB/core @307 predicts 273.5 us ~= the 276.3
  baseline.  Every MB saved ~= 3.3 us.
- t32 (4x32-row PSUM bands on DVE+ACT) sims at 346 us: per-band PSUM access
  (120/172 cyc) + per-op overhead blow the engines up.  Instead "t64r" keeps
  the two cheap [64,160] PSUM reads and shears AFTERWARD in SBUF, where
  fp16 copies run at DVE 2x/4x rate and Pool may assist (Pool cannot read
  PSUM, but SBUF->SBUF is legal): bands b0,b1 on DVE, b2 on ACT (its own
  rows), b3 on Pool, each as ONE strided-block op per y covering all 5
  tiles (per-tile ops serialize on sem waits: 340 us -> 258 us sim).
  Stores drop 18.9 -> 15.7MB (t32 host layout).
- Stores issue from the Pool queue (SWDGE, stq=gpsimd): on ACT they park
  the in-order seq behind the repack sems and bubble the DMA pool.
- f32r: float32r-bitcast matmuls with a 256-wide moving window (cols >= 224
  junk; lcontig packs lt rows so the window fits) stream at 1 cyc/row vs
  fp32's 4: PE 184 -> 55 us, shortening the compute-paced drain tail.
  Final sim 233.1 us with 3.1MB fewer bytes than t64 (233.7); projected
  ~260-270 on HW vs t64's 276.
- lcontig: left rows packed back-to-back (zero pad once at tile end, rhs
  windows read next-row junk the host crops) merges left-load descriptors
  to 5120B/partition like the right plane.
- Measurement regime matters: R=513 bursts (~0.14s device time per call)
  match the 276 us baseline; R=32769 (~9.4s sustained) progressively
  throttles to ~460-475 ns/iter (power/DVFS), drifting upward run to run.
  test.py uses R=4097 (~1.1s bursts, still pre-throttle; sigma ~30us).
- DMA descriptor size is the missing 15% of bandwidth: loads-only probes
  measured 209.1 us with mixed 2560B/5120B descriptors vs 195.0 us with
  all-5120B (lcontig) -- ~327-344 B/ns at 5120B vs ~267-286 at 2560B.
  Hence lcontig for the left plane and spair=2 (two y-pairs per stage
  tile/store, 5120B store descriptors); both validated at rel err
  9.298e-03.  fp32r matmuls (1 cyc/row at free-dim >= 256, PE 184 -> 55us)
  compile once inputs are declared float32r (verifier demands a rounded
  producer; Memset can't emit f32r, but the pad row may stay unwritten as
  it only feeds host-cropped junk) -- but HW numerics are tf32-grade:
  rel err 5.76.  DEAD END under the 2e-2 gate; do not retry.
"""

import sys

if "/opt/trn_rl_repo" not in sys.path:
    sys.path.insert(0, "/opt/trn_rl_repo")

import numpy as np

B, C, H, W = 4, 128, 192, 640
D = 96
HSH = H // 2          # rows per core

MODE = "t64r"   # "base" (M=64 fp32 PE) | "m128" | "split3" | "t32" | "t64" | "t64r"
# Shipped config: two-stage evacuation with stores issued from the Pool
# SWDGE queue, repack bands DVE/DVE/ACT/Pool, contiguous left loads and
# two-y-pair stores (both 5120B/partition DMA descriptors: measured 2560B
# descriptors stream ~267-286 B/ns on HW vs ~327-344 for 5120B).
# HW-validated rel err 9.298e-03 end-to-end in exactly this config.
# Sim 239.4 us; HW projection ~246 us vs the 276.3 us t64 baseline
# (loads-only probes: 209.1 us mixed-desc, 195.0 us lcontig).
BUILD_KW = {"stq": "gpsimd", "repack": "vvag", "lcontig": True, "spair": 2}


def _geom(mode):
    # t64 computes with M=128 matmuls but stores the baseline M=64 layout
    # (the band shift happens during PSUM evacuation), so its host-visible
    # geometry is the M=64 one.
    mt = 128 if mode in ("m128", "t32", "t64r") else 64   # M (x') tile height
    nw = mt + 96                         # free (x) tile width per matmul
    nt = W // mt                         # x'-tiles per row
    st = 128 // mt                       # tiles stacked per psum tile
    nps = nt // st                       # psum tiles per row
    return mt, nw, nt, st, nps


MT, NW, NT, ST, NPS = _geom(MODE)

_compiled = None

# consumed by bench.py only (not by the grading harness)
if MODE == "split3":
    BENCH_INPUTS = [(n, (C, HSH, W), "float16")
                    for n in ("left_hi", "left_lo", "right_hi", "right_lo")]
else:
    BENCH_INPUTS = [("left", (C, HSH, W), "float32"),
                    ("right", (C, HSH, W), "float32")]


CROP4 = True          # t64: store last col block 128 wide (x >= W junk cut)


def _build(repeat=1, mode=None, yb=2, lbufs=4, sbufs=4, pbufs=8, engsel=3,
           ldq="scalar", stq="sync", crop4=None, scr_f32=False, probe=None,
           abufs=6, repack="vvgg", lcontig=False, stsplit=False, f32r=False,
           spair=1):
    # spair=2: group two y-pairs per stage tile/store so store descriptors
    # reach 5120B/partition (2560B descriptors move ~267 B/ns on HW vs ~344
    # for 5120B -- measured by the loads-only lcontig A/B probe).
    # f32r: stream the matmuls as float32r with a 256-wide moving window
    # (cols 224-255 are never-read junk).  fp32r at free-dim >= 256 runs the
    # PE at 1 cyc/row vs plain fp32's 4 (cost model + HW scan); numerics are
    # validated empirically against the 2e-2 gate.  Requires lcontig (the
    # packed lt rows leave room for the wider window).
    # lcontig: store left rows back-to-back (pad row at the END of the tile
    # instead of 96 cols after every row) so left-load DMA descriptors merge
    # to one contiguous 5120B/partition element, matching the right plane.
    # rhs windows then run into the next row's data instead of zeros; those
    # columns are x >= W junk the host never reads (finite values, f16-safe).
    # probe="loads": issue only the input DMAs (plus a tiny anchor copy+store
    # per y-pair so nothing is dead) — isolates pure load-bandwidth on HW.
    # pbufs=8 (all PSUM banks) verified bitwise-identical on device and
    # -0.4us in sim vs 6
    # scr_f32=True reconstructs the original session's fp32-scratch baseline
    # (timing comparisons only; the host path always reads fp16 scratch)
    if mode is None:
        mode = MODE
    if crop4 is None:
        crop4 = CROP4
    import contextlib
    import concourse.bacc as bacc
    import concourse.tile as tile
    import concourse.mybir as mybir

    mt, nw, nt, st_, nps = _geom(mode)
    split = mode == "split3"
    t32 = mode == "t32"
    t64 = mode == "t64"
    t64r = mode == "t64r"
    if t64r:
        # two-stage evacuation: DVE/ACT move the two 64-row bands into a
        # [128, 160] fp16 staging tile (one PSUM read each, as t64), then
        # cheap fp16 SBUF->SBUF copies (DVE 2x/4x mode + Pool, which may not
        # read PSUM but may read SBUF) shear the four 32-row sub-bands into
        # the packed 128-wide t32 layout -- 15.7MB stores instead of 18.9MB.
        mt, nw, nt, st_, nps = 128, 224, 5, 1, 5
        if ldq == "scalar":
            ldq = "sync"
            if stq == "sync":
                stq = "scalar"
    if t64 and ldq == "scalar":
        # SP must be a pure load driver and ACT issues stores after its own
        # band copies; anything else idles the DMA pool ~9% (measured in sim)
        ldq = "sync"
        if stq == "sync":
            stq = "scalar"
    if t64:
        # PE runs M=128 tiles; evacuation splits each [128, 224] psum tile
        # into two 64-row bands shifted by 64 cols, reproducing the M=64
        # rect layout (stores 160 cols/band instead of 224).
        mt, nw, nt, st_, nps = 128, 224, 5, 1, 5

    nc = bacc.Bacc("TRN2", target_bir_lowering=False, debug=False, num_devices=8)
    f32 = mybir.dt.float32
    f16 = mybir.dt.float16
    in_dt = f16 if split else f32
    if f32r:
        assert lcontig and mode in ("t64", "t64r")
        nwr = 256       # moving window streamed by the PE
        # the walrus birverifier requires every producer feeding an FP32r
        # matmult to emit FP32r-rounded data; the input DMA is the producer,
        # so the DRAM tensors and SBUF tiles are declared float32r outright
        # (same 4-byte payload -- the PE rounds internally)
        in_dt = mybir.dt.float32r
    else:
        nwr = nw

    if split:
        in_aps = {
            n: nc.dram_tensor(n, [C, HSH, W], f16, kind="ExternalInput").ap()
            for n in ("left_hi", "left_lo", "right_hi", "right_lo")
        }
    else:
        in_aps = {
            n: nc.dram_tensor(n, [C, HSH, W], in_dt, kind="ExternalInput").ap()
            for n in ("left", "right")
        }
    # y-pair-major, p-major layout: one store covers 2 rows as a single
    # plain contiguous-per-partition DMA; float16 payload.  In t32 mode the
    # per-y row is [5 blocks x 128 cols] of 32-row bands (see below) instead
    # of [NPS x NW] rect tiles.
    if t32 or t64r:
        rw_scr = 5 * 128
    elif t64:
        rw_scr = 4 * 160 + 128 if crop4 else 5 * 160
    else:
        rw_scr = nps * nw
    if spair != 1:
        assert mode == "t64r" and probe is None and not stsplit
    scr_dt = f32 if scr_f32 else f16
    scr_ap = nc.dram_tensor(
        "scr", [HSH // (2 * spair), 128, spair * 2 * rw_scr], scr_dt,
        kind="ExternalOutput"
    ).ap()

    WPAD = W + 96  # L is zero-padded on the right so every rhs window is full
    YB = yb        # y rows loaded per input DMA

    lplanes = ["left_hi", "left_lo"] if split else ["left"]
    rplanes = ["right_hi", "right_lo"] if split else ["right"]

    with tile.TileContext(nc) as tc:
        with (
            tc.tile_pool(name="lpool", bufs=lbufs) as lpool,
            tc.tile_pool(name="rpool", bufs=lbufs) as rpool,
            tc.tile_pool(name="stage", bufs=sbufs) as stage_pool,
            tc.tile_pool(name="apool", bufs=abufs) as apool,
            tc.tile_pool(name="psum", bufs=pbufs, space="PSUM") as psum_pool,
        ):
            rep_ctx = (
                tc.For_i(0, repeat, 1) if repeat > 1 else contextlib.nullcontext()
            )
            with rep_ctx:
                for y0 in range(0, HSH, YB):
                    # [c, (y pair, x)] input tiles; loads on the ACT HWDGE
                    # ring so they round-robin against stores on the SP ring
                    ldeng = getattr(nc, ldq)
                    # First iteration: issue per-row half loads so the first
                    # matmul's operands land in ~half the time (single-shot
                    # exec time includes this warmup).
                    nld = 2 if (y0 == 0 and repeat == 1) else 1
                    lts, rts = {}, {}
                    views = []
                    for n in lplanes:
                        if lcontig:
                            lt = lpool.tile([128, (YB + 1) * W], in_dt,
                                            name=f"lt_{n}_{y0}", tag=f"lt_{n}")
                            lt3 = lt.rearrange("c (y w) -> c y w", y=YB + 1)
                            views.append((lt3[:, 0:YB, 0:W], in_aps[n]))
                            if not f32r:
                                # f32r: codegen rejects float32r Memset, and
                                # the pad only feeds x >= W junk columns the
                                # host crops -- garbage there is harmless, so
                                # the pad row is simply left unwritten.
                                nc.vector.memset(lt3[:, YB, 0:nwr - mt], 0.0)
                        else:
                            lt = lpool.tile([128, YB * WPAD], in_dt,
                                            name=f"lt_{n}_{y0}", tag=f"lt_{n}")
                            lt3 = lt.rearrange("c (y w) -> c y w", y=YB)
                            views.append((lt3[:, :, 0:W], in_aps[n]))
                            nc.vector.memset(lt3[:, :, W:WPAD], 0.0)
                        lts[n] = lt
                    for n in rplanes:
                        rt = rpool.tile([128, YB * W], in_dt, name=f"rt_{n}_{y0}",
                                        tag=f"rt_{n}")
                        views.append((rt.rearrange("c (y w) -> c y w", y=YB),
                                      in_aps[n]))
                        rts[n] = rt
                    # interleave per-row halves (lt_h0, rt_h0, lt_h1, rt_h1)
                    # so the first matmul's operands land first
                    for h in range(nld):
                        lo, hi = h * YB // nld, (h + 1) * YB // nld
                        for v3, ap in views:
                            ldeng.dma_start(v3[:, lo:hi], ap[:, y0 + lo:y0 + hi, :])

                    if probe == "loads":
                        stg = stage_pool.tile([128, 32], scr_dt,
                                              name=f"st_{y0}", tag="st")
                        nc.vector.tensor_copy(stg[:], lts[lplanes[0]][:, 0:32])
                        getattr(nc, stq).dma_start(scr_ap[y0 // 2, :, 0:32], stg[:])
                        continue

                    if split:
                        # G = r_hi l_hi + r_hi l_lo + r_lo l_hi (lo*lo dropped,
                        # ~2^-22 relative) accumulated in fp32 PSUM
                        passes = [
                            (rts["right_hi"], lts["left_hi"]),
                            (rts["right_hi"], lts["left_lo"]),
                            (rts["right_lo"], lts["left_hi"]),
                        ]
                    else:
                        passes = [(rts["right"], lts["left"])]

                    RW = rw_scr    # per-row stage width
                    if (y0 // YB) % spair == 0:
                        stg = stage_pool.tile([128, spair * 2 * RW], scr_dt,
                                              name=f"st_{y0}", tag="st")
                    for yi in range(YB):
                        for s in range(nps):
                            ps = psum_pool.tile([128, nwr], f32,
                                                name=f"ps_{y0 + yi}_{s}", tag="ps")
                            for u in range(st_):
                                t = st_ * s + u
                                q0 = yi * (W if lcontig else WPAD) + mt * t
                                for pi, (rt, lt) in enumerate(passes):
                                    lv = rt[:, yi * W + mt * t: yi * W + mt * t + mt]
                                    rv = lt[:, q0: q0 + nwr]
                                    if f32r:
                                        lv = lv.bitcast(mybir.dt.float32r)
                                        rv = rv.bitcast(mybir.dt.float32r)
                                    nc.tensor.matmul(
                                        ps[mt * u: mt * (u + 1), :],
                                        lhsT=lv,
                                        rhs=rv,
                                        start=(pi == 0),
                                        stop=(pi == len(passes) - 1),
                                        tile_position=None if st_ == 1 else (0, mt * u),
                                    )
                            if t64r:
                                # stage A: one PSUM read per engine (as t64),
                                # into a per-y [128, 5*160] staging tile
                                if s == 0:
                                    stga = apool.tile([128, nps * 160], scr_dt,
                                                      name=f"sa_{y0 + yi}",
                                                      tag="sa")
                                nc.vector.tensor_copy(
                                    stga[0:64, 160 * s: 160 * s + 160],
                                    ps[0:64, 0:160])
                                nc.scalar.copy(
                                    stga[64:128, 160 * s: 160 * s + 160],
                                    ps[64:128, 64:224])
                                if s == nps - 1:
                                    # stage B: shear the four 32-row sub-bands
                                    # into the packed 128-wide layout, one
                                    # strided-block op per band covering all 5
                                    # tiles (amortizes per-op overhead and sem
                                    # waits).  stga row p holds psum cols
                                    # [64*(p//64), +160); band b needs psum
                                    # cols [32b, 32b+128) -> stga cols
                                    # [32b % 64, +128) of each 160-block.
                                    sa3 = stga.rearrange("c (s w) -> c s w",
                                                         s=nps)
                                    st3 = stg.rearrange("c (y s w) -> c y s w",
                                                        y=YB * spair, s=nps)
                                    yst = ((y0 // YB) % spair) * YB + yi
                                    for b, e in enumerate(repack):
                                        c0 = 32 * (b % 2)
                                        src = sa3[32 * b: 32 * b + 32, :,
                                                  c0: c0 + 128]
                                        dst = st3[32 * b: 32 * b + 32, yst]
                                        if e == "v":
                                            nc.vector.tensor_copy(dst, src)
                                        elif e == "a":
                                            nc.scalar.copy(dst, src)
                                        else:
                                            nc.gpsimd.tensor_copy(dst, src)
                            elif t64:
                                # two 64-row bands, cols [64g, 64g+160), into
                                # the M=64-rect stage layout.  GPSIMD cannot
                                # read PSUM (walrus birverifier), so the bands
                                # go to DVE + ACT; loads then issue from the
                                # SP queue so ACT's copy waits never delay
                                # load dma_starts (in-order sequencer).
                                bw = 128 if (crop4 and s == 4) else 160
                                blk = yi * rw_scr + s * 160
                                for g in range(2):
                                    src = ps[64 * g: 64 * g + 64,
                                             64 * g: 64 * g + bw]
                                    dst = stg[64 * g: 64 * g + 64,
                                              blk: blk + bw]
                                    if g == 0:
                                        nc.vector.tensor_copy(dst, src)
                                    else:
                                        nc.scalar.copy(dst, src)
                            elif t32:
                                # Tight-band evacuation: the [128, 224] rect
                                # tile holds, for partition p (x' = 128s+p),
                                # useful cols f in [p, p+96).  Store only the
                                # 32-row band windows [32g, 32g+128) -- junk
                                # 25% instead of rect's 43% -- with the
                                # per-band shift folded into three otherwise
                                # idle engines' copies (a full per-partition
                                # shear is not expressible in any single op).
                                blk = (yi * 5 + s) * 128
                                for g in range(4):
                                    src = ps[32 * g: 32 * g + 32,
                                             32 * g: 32 * g + 128]
                                    dst = stg[32 * g: 32 * g + 32,
                                              blk: blk + 128]
                                    eng = (s + g) % engsel
                                    if eng == 0:
                                        nc.vector.tensor_copy(dst, src)
                                    elif eng == 1:
                                        nc.scalar.copy(dst, src)
                                    else:
                                        nc.gpsimd.tensor_copy(dst, src)
                            else:
                                nc.vector.tensor_copy(
                                    stg[:, yi * RW + s * nw: yi * RW + (s + 1) * nw],
                                    ps[:],
                                )
                    # one plain contiguous store per y-pair; the right-edge
                    # junk of the last psum tile block rides along (host
                    # never reads it) -- keeping the AP trivial.  (Per-y
                    # stores everywhere regress +2.5us -- doubled issue/sem
                    # overhead on the ACT sequencer -- but splitting only the
                    # FINAL y-pair overlaps the drain: the y=94 half issues
                    # while y=95's bands still copy.)
                    last = (t64 or t64r) and repeat == 1 and y0 == HSH - YB \
                        and spair == 1
                    if stsplit or last:
                        for yi in range(YB):
                            getattr(nc, stq).dma_start(
                                scr_ap[y0 // 2, :, yi * rw_scr:(yi + 1) * rw_scr],
                                stg[:, yi * rw_scr:(yi + 1) * rw_scr],
                            )
                    elif (y0 // YB) % spair == spair - 1:
                        getattr(nc, stq).dma_start(
                            scr_ap[y0 // (YB * spair)], stg[:])

    nc.compile()
    return nc


def _host_index(mode=None):
    """idx[d, x] -> flat offset into scr[y] holding G[x-d, x].

    Valid only where x >= d; mask handles the rest.
    """
    if mode is None:
        mode = MODE
    d = np.arange(D)[:, None]
    x = np.arange(W)[None, :]
    xp = np.maximum(x - d, 0)        # x' = x - d
    if mode in ("t32", "t64r"):
        # scr[y] is [128 p, 5 s, 128 j]: band (s, g) holds x' = 128s + 32g + q
        # (partition p = 32g + q) at j = x - 128s - 32g; j = d + (x'%32) < 128
        s = xp // 128
        p = xp - 128 * s
        g = p // 32
        j = x - 128 * s - 32 * g
        idx = (p * 5 + s) * 128 + j
    else:
        t = xp // MT                 # x'-tile
        q = xp - MT * t              # row within tile
        s = t // ST                  # psum tile
        u = t - ST * s               # col group within psum tile
        f = x - MT * t               # col within tile (< NW always)
        p = MT * u + q               # psum partition
        rw = _scr_rw()               # scr[y] is [128 p, rw] with blocks at NW*s
        idx = p * rw + s * NW + f
    mask = (x >= d)
    return idx.astype(np.int64), mask


def _scr_rw(mode=None):
    if mode is None:
        mode = MODE
    if mode in ("t32", "t64r"):
        return 5 * 128
    if mode == "t64" and CROP4:
        return 4 * 160 + 128
    return NPS * NW


def kernel(left, right, num_disparities):
    global _compiled
    left = np.asarray(left)
    right = np.asarray(right)
    assert int(num_disparities) == D
    assert left.shape == (B, C, H, W) and right.shape == (B, C, H, W)

    if _compiled is None:
        _compiled = _build(**BUILD_KW)
    nc = _compiled

    from concourse.bass_utils import run_bass_kernel_spmd

    split = MODE == "split3"
    if split:
        lh = left.astype(np.float16)
        ll = (left - lh.astype(np.float32)).astype(np.float16)
        rh = right.astype(np.float16)
        rl = (right - rh.astype(np.float32)).astype(np.float16)

    in_maps = []
    for k in range(8):
        b, hh = k // 2, k % 2
        sl = slice(96 * hh, 96 * hh + 96)
        if split:
            in_maps.append({
                "left_hi": np.ascontiguousarray(lh[b, :, sl, :]),
                "left_lo": np.ascontiguousarray(ll[b, :, sl, :]),
                "right_hi": np.ascontiguousarray(rh[b, :, sl, :]),
                "right_lo": np.ascontiguousarray(rl[b, :, sl, :]),
            })
        else:
            in_maps.append({
                "left": np.ascontiguousarray(left[b, :, sl, :]),
                "right": np.ascontiguousarray(right[b, :, sl, :]),
            })

    res = run_bass_kernel_spmd(nc, in_maps, list(range(8)))

    idx, mask = _host_index()
    out = np.zeros((B, D, H, W), dtype=np.float32)
    for k in range(8):
        b, hh = k // 2, k % 2
        # scr is [y-slabs, 128 p, 2*spair*rw] f16; un-slab to [96, 128*rw]
        rw = _scr_rw()
        sp = BUILD_KW.get("spair", 1)
        scr = (
            res.results[k]["scr"]
            .reshape(HSH // (2 * sp), 128, 2 * sp, rw)
            .swapaxes(1, 2)
            .reshape(HSH, 128 * rw)
        )
        gathered = scr[:, idx.ravel()].astype(np.float32).reshape(HSH, D, W)
        gathered *= mask[None, :, :]
        out[b, :, 96 * hh: 96 * hh + 96, :] = gathered.transpose(1, 0, 2)
    return out

